# revision 24
# baseline (speedup 1.0000x reference)
"""Bass/Trainium2 kernel for BertLSHSelfAttention (B=2, S=1024, D=768, H=12).

Self-contained: accepts FULL unsharded inputs, shards (batch, head) pairs
across 8 NeuronCores (3 heads per core), runs a Tile/Bass program per core,
and reassembles the full [B, S, D] output.

Per-core pipeline:
  1. fp16 QKV projections on TensorE (packed 576 output rows in 5 M-groups).
  2. fp32 LSH hash projections via host-folded rv_eff = rv @ W (sign fidelity),
     hash values via block-diag coeff matmul, mod 64 on VectorE.
  3. One-hot bucket encodings (bf16) via selector-matmul broadcast + is_equal.
  4. Collision counts = onehot_q @ onehot_k^T (bf16, exact), masked scores
     X = min(cnt,1) * (q @ k^T fp16), triu symmetrization via PE transposes.
  5. exp((1/8) A) on ScalarE with fused row-sum accumulation -> fp16 E.
  6. ctx = (E @ v) * (1/Z) with fp16 PV matmul (E symmetric => no transpose).
"""

import math
from contextlib import ExitStack

import numpy as np
import ml_dtypes

import concourse.bass as bass
import concourse.tile as tile
from concourse import bacc, mybir

B, S, D = 2, 1024, 768
H, NH, BANDS, TABLE = 12, 4, 8, 64
dh = D // H            # 64
N_CORES = 8
HPC = 3                # heads per core
KCH = D // 128         # 6 contraction chunks
NBI = S // 128         # 8 row blocks
F16 = mybir.dt.float16
F32 = mybir.dt.float32
BF16 = mybir.dt.bfloat16
AOT = mybir.AluOpType
ACT = mybir.ActivationFunctionType

_CACHE = {}


# --------------------------------------------------------------------------
# host-side prep
# --------------------------------------------------------------------------

def _core_heads(c):
    return [(c % 4) * HPC + j for j in range(HPC)]


def _host_prep(inputs):
    """Build per-core input maps (numpy) for the SPMD program."""
    hs = np.asarray(inputs["hidden_states"], np.float32)
    wq = np.asarray(inputs["w_q"], np.float32)
    wk = np.asarray(inputs["w_k"], np.float32)
    wv = np.asarray(inputs["w_v"], np.float32)
    bq = np.asarray(inputs["b_q"], np.float32)
    bk = np.asarray(inputs["b_k"], np.float32)
    bv = np.asarray(inputs["b_v"], np.float32)
    rv = np.asarray(inputs["random_vectors"], np.float32)   # [B,H,NH,BANDS,dh]
    co = np.asarray(inputs["hash_coeffs"])                  # [B,H,NH,BANDS] int32

    # constants shared by all cores
    e8 = np.zeros((8, 4 * 128), ml_dtypes.bfloat16)
    for j in range(4):
        for p in range(128):
            e8[2 * j + p // 64, j * 128 + p] = 1.0
    e8_rep = np.zeros((72, 4 * 128), ml_dtypes.bfloat16)
    for base in (0, 32, 64):
        e8_rep[base:base + 8] = e8
    e8 = e8_rep
    iota2 = (np.arange(128) % 64).astype(np.float32)[:, None]
    pp, ff = np.mgrid[0:128, 0:128]
    triu = (pp <= ff).astype(np.float16)
    slow = (pp > ff).astype(np.uint8)
    ident32 = np.eye(128, dtype=np.float32)
    ident16 = np.eye(128, dtype=np.float16)

    in_maps = []
    for c in range(N_CORES):
        b = c // 4
        heads = _core_heads(c)
        hT = hs[b].T.copy()                                  # [768, 1024]
        # packed [128, KCH*S]: col block k holds rows k*128..k*128+127
        hT16 = np.ascontiguousarray(
            hT.astype(np.float16).reshape(KCH, 128, S).transpose(1, 0, 2)
        ).reshape(128, KCH * S)
        hT32 = np.ascontiguousarray(
            hT.astype(np.float32).reshape(KCH, 128, S).transpose(1, 0, 2)
        ).reshape(128, KCH * S)

        # packed projection weights: rows = per-head [q(64) k(64) v(64)]
        W_all = np.concatenate(
            [w[h * dh:(h + 1) * dh] for h in heads for w in (wq, wk, wv)], 0)
        b_all = np.concatenate(
            [bb[h * dh:(h + 1) * dh] for h in heads for bb in (bq, bk, bv)], 0)
        W_pad = np.zeros((640, D), np.float32)
        W_pad[:576] = W_all
        b_pad = np.zeros(640, np.float32)
        b_pad[:576] = b_all
        lhsT = W_pad.T                                       # [768, 640]
        # packed [128, 5*KCH*128]: col block (g*KCH+k)*128 holds chunk (g,k)
        w16 = np.zeros((128, 5 * KCH * 128), np.float16)
        for g in range(5):
            for k in range(KCH):
                w16[:, (g * KCH + k) * 128:(g * KCH + k + 1) * 128] = \
                    lhsT[k * 128:(k + 1) * 128, g * 128:(g + 1) * 128]
        b_pack = np.ascontiguousarray(b_pad.reshape(5, 128).T).astype(np.float32)

        # hash-projection weights: rv folded through W in float64
        rveff = np.zeros((128, 2 * KCH * 128), np.float32)
        rvbias = np.zeros((128, 2), np.float32)
        cb = np.zeros((128, HPC * 32), ml_dtypes.bfloat16)
        for slot, h in enumerate(heads):
            g2, half = slot // 2, slot % 2
            rq = np.einsum("nkd,dc->nkc", rv[b, h].astype(np.float64),
                           wq[h * dh:(h + 1) * dh].astype(np.float64))
            rk = np.einsum("nkd,dc->nkc", rv[b, h].astype(np.float64),
                           wk[h * dh:(h + 1) * dh].astype(np.float64))
            blk = np.concatenate([rq.reshape(32, D), rk.reshape(32, D)], 0)
            blkT = blk.T.astype(np.float32).reshape(KCH, 128, 64)
            for k in range(KCH):
                rveff[:, (g2 * KCH + k) * 128 + half * 64:
                      (g2 * KCH + k) * 128 + half * 64 + 64] = blkT[k]
            bq_f = np.einsum("nkd,d->nk", rv[b, h].astype(np.float64),
                             bq[h * dh:(h + 1) * dh].astype(np.float64)).ravel()
            bk_f = np.einsum("nkd,d->nk", rv[b, h].astype(np.float64),
                             bk[h * dh:(h + 1) * dh].astype(np.float64)).ravel()
            rvbias[half * 64:half * 64 + 32, g2] = bq_f.astype(np.float32)
            rvbias[half * 64 + 32:half * 64 + 64, g2] = bk_f.astype(np.float32)
            for n in range(NH):
                for band in range(BANDS):
                    r0 = half * 64
                    cval = float(co[b, h, n, band])
                    cb[r0 + n * 8 + band, slot * 32 + n] = cval        # hq
                    cb[r0 + 32 + n * 8 + band, slot * 32 + 4 + n] = cval  # hk

        in_maps.append({
            "hT16": hT16, "hT32": hT32,
            "w16": w16, "b_pack": b_pack,
            "rveff": rveff, "rvbias": rvbias, "cb": cb,
            "e8": e8, "iota2": iota2, "triu": triu, "slow": slow,
            "ident32": ident32, "ident16": ident16,
        })
    return in_maps


# --------------------------------------------------------------------------
# device program
# --------------------------------------------------------------------------

def _declare_io(nc):
    aps = {}
    decl = [
        ("hT16", [128, KCH * S], F16), ("hT32", [128, KCH * S], F32),
        ("w16", [128, 5 * KCH * 128], F16), ("b_pack", [128, 5], F32),
        ("rveff", [128, 2 * KCH * 128], F32), ("rvbias", [128, 2], F32),
        ("cb", [128, HPC * 32], BF16), ("e8", [72, 4 * 128], BF16),
        ("iota2", [128, 1], F32), ("triu", [128, 128], F16),
        ("slow", [128, 128], mybir.dt.uint8), ("ident32", [128, 128], F32),
        ("ident16", [128, 128], F16),
    ]
    for name, shape, dt in decl:
        aps[name] = nc.dram_tensor(name, shape, dt, kind="ExternalInput").ap()
    aps["out"] = nc.dram_tensor("out", [HPC, S, dh], F32,
                                kind="ExternalOutput").ap()
    return aps


def _emit_body(tc, aps, ctx):
    nc = tc.nc
    const = ctx.enter_context(tc.tile_pool(name="const", bufs=1))
    sbuf = ctx.enter_context(tc.tile_pool(name="sbuf", bufs=1))

    # ---- constants / inputs to SBUF ----
    def load(name, shape, dt, src):
        t = const.tile(shape, dt, name=name)
        nc.sync.dma_start(t[:], src)
        return t

    hT16_t = load("hT16_t", [128, KCH * S], F16, aps["hT16"][:])
    w16_t = load("w16_t", [128, 5 * KCH * 128], F16, aps["w16"][:])
    hT32_t = load("hT32_t", [128, KCH * S], F32, aps["hT32"][:])
    rveff_t = load("rveff_t", [128, 2 * KCH * 128], F32, aps["rveff"][:])
    hT16 = [hT16_t[:, k * S:(k + 1) * S] for k in range(KCH)]
    hT32 = [hT32_t[:, k * S:(k + 1) * S] for k in range(KCH)]
    w16 = [[w16_t[:, (g * KCH + k) * 128:(g * KCH + k + 1) * 128]
            for k in range(KCH)] for g in range(5)]
    rveff = [[rveff_t[:, (g * KCH + k) * 128:(g * KCH + k + 1) * 128]
              for k in range(KCH)] for g in range(2)]
    b_pack_t = load("b_pack_t", [128, 5], F32, aps["b_pack"][:])
    b_pack = [b_pack_t[:, g:g + 1] for g in range(5)]
    rvbias_t = load("rvbias_t", [128, 2], F32, aps["rvbias"][:])
    rvbias = [rvbias_t[:, g:g + 1] for g in range(2)]
    cb_t = load("cb_t", [128, HPC * 32], BF16, aps["cb"][:])
    e8_t = load("e8_t", [72, 4 * 128], BF16, aps["e8"][:])
    cb = [cb_t[:, j * 32:(j + 1) * 32] for j in range(HPC)]
    iota2 = load("iota2", [128, 1], F32, aps["iota2"][:])
    triu = load("triu", [128, 128], F16, aps["triu"][:])
    slow = load("slow", [128, 128], mybir.dt.uint8, aps["slow"][:])
    ident32 = load("ident32", [128, 128], F32, aps["ident32"][:])
    ident16 = load("ident16", [128, 128], F16, aps["ident16"][:])

    # ---- persistent per-head tensors ----
    qT = [sbuf.tile([64, S], F16, name=f"qT_{h}") for h in range(HPC)]
    kT = [sbuf.tile([64, S], F16, name=f"kT_{h}") for h in range(HPC)]
    vT = [sbuf.tile([64, S], F16, name=f"vT_{h}") for h in range(HPC)]
    vN = [sbuf.tile([128, NBI * dh], F16, name=f"vN_{h}") for h in range(HPC)]
    bits = [sbuf.tile([128, S], BF16, name=f"bits_{g}") for g in range(2)]
    hqi = sbuf.tile([96, S], mybir.dt.int32, name="hqi")
    hqa = sbuf.tile([96, S], mybir.dt.int32, name="hqa")
    hq8_all = sbuf.tile([96, S], BF16, name="hq8_all")
    # one-hot chunks per head: [qA, qB, kA, kB]
    oh = [[sbuf.tile([128, S], BF16, name=f"oh_{h}_{j}") for j in range(4)]
          for h in range(HPC)]

    # ---- phase 1: QKV projections (fp16) ----
    # group g covers packed rows [g*128, g*128+128); row layout per head:
    # q(64) k(64) v(64).  seg targets: (tile, tile_row_base)
    seg_map = {
        0: [(qT[0], 0), (kT[0], 0)],
        1: [(vT[0], 0), (qT[1], 0)],
        2: [(kT[1], 0), (vT[1], 0)],
        3: [(qT[2], 0), (kT[2], 0)],
        4: [(vT[2], 0), (None, 0)],
    }
    with tc.tile_pool(name="psum1", bufs=2, space="PSUM") as pp1, \
         tc.tile_pool(name="psum2", bufs=2, space="PSUM") as pp2:
        for g in range(5):
            for nh2 in range(2):
                c0 = nh2 * 512
                proj_ps = pp1.tile([128, 512], F32, name="proj_ps")
                for k in range(KCH):
                    nc.tensor.matmul(proj_ps[:], w16[g][k][:],
                                     hT16[k][:, c0:c0 + 512],
                                     start=(k == 0), stop=(k == KCH - 1))
                for half in range(2):
                    tgt, _ = seg_map[g][half]
                    if tgt is None:
                        continue
                    nc.scalar.activation(
                        tgt[:, c0:c0 + 512],
                        proj_ps[half * 64:half * 64 + 64, :],
                        ACT.Identity,
                        bias=b_pack[g][half * 64:half * 64 + 64, :])
        # v natural layout via PE transpose
        for h in range(HPC):
            for b4 in range(NBI // 4):
                vtr_ps = pp1.tile([128, 256], F16, name="vtr_ps", bufs=2)
                for q in range(4):
                    bi = b4 * 4 + q
                    nc.tensor.transpose(vtr_ps[:, q * 64:q * 64 + 64],
                                        vT[h][:, bi * 128:bi * 128 + 128],
                                        ident16[0:64, 0:64])
                nc.vector.tensor_copy(vN[h][:, b4 * 256:b4 * 256 + 256],
                                      vtr_ps[:])

        # ---- phase 2: hashes ----
        for g in range(2):
            for nh2 in range(2):
                c0 = nh2 * 512
                hash_ps = pp2.tile([128, 512], F32, name="hash_ps")
                for k in range(KCH):
                    nc.tensor.matmul(hash_ps[:], rveff[g][k][:],
                                     hT32[k][:, c0:c0 + 512],
                                     start=(k == 0), stop=(k == KCH - 1))
                nc.vector.tensor_scalar(
                    out=bits[g][:, c0:c0 + 512], in0=hash_ps[:],
                    scalar1=rvbias[g][:], scalar2=0.0,
                    op0=AOT.add, op1=AOT.is_gt)
        hq_ps = pp2.tile([96, S], F32, name="hq_ps", bufs=1)
        for slot in range(HPC):
            g2 = slot // 2
            for nh2 in range(2):
                c0 = nh2 * 512
                nc.tensor.matmul(hq_ps[32 * slot:32 * slot + 32, c0:c0 + 512],
                                 cb[slot][:], bits[g2][:, c0:c0 + 512],
                                 start=True, stop=True)
        # hash mod 64 == int bitwise AND with 63 (raw hash is an exact int)
        nc.vector.tensor_copy(hqi[:], hq_ps[:])
        nc.vector.tensor_scalar(out=hqa[:], in0=hqi[:], scalar1=63,
                                scalar2=None, op0=AOT.bitwise_and)
        nc.vector.tensor_copy(hq8_all[:], hqa[:])
    # ---- phase 3: attention per head ----
    att = ctx.enter_context(tc.tile_pool(name="att", bufs=1))
    with tc.tile_pool(name="psum3", bufs=1, space="PSUM") as pp3:
        for h in range(HPC):
            # one-hot encodings: selector-matmul broadcast + DVE is_equal
            for j in range(4):
                for nh2 in range(2):
                    c0 = nh2 * 512
                    bc_ps = pp3.tile([128, 512], F32, name="bc_ps", tag="ssc",
                                     bufs=2)
                    nc.tensor.matmul(
                        bc_ps[:],
                        e8_t[32 * h:32 * h + 8, j * 128:(j + 1) * 128],
                        hq8_all[32 * h:32 * h + 8, c0:c0 + 512],
                        start=True, stop=True)
                    nc.vector.tensor_scalar(
                        out=oh[h][j][:, c0:c0 + 512], in0=bc_ps[:],
                        scalar1=iota2[:], scalar2=None, op0=AOT.is_equal)
            E = [att.tile([128, S], F16, name=f"E_{h}_{bi}", tag=f"E{bi}",
                          bufs=2) for bi in range(NBI)]
            Us = [att.tile([128, S], F16, name=f"U_{h}_{bi}", tag=f"U{bi}",
                           bufs=2) for bi in range(NBI)]
            zparts = att.tile([128, 64], F32, name=f"zp_{h}", tag="zp", bufs=2)
            nc.vector.memset(zparts[:], 0.0)
            zr = att.tile([128, NBI], F32, name=f"zr_{h}", tag="zr", bufs=2)

            # pass A: masked upper strips + diag + upper exp
            for bi in range(NBI):
                U = Us[bi]
                c0 = bi * 128
                ntiles = ([(c0, 512 - c0), (512, 512)] if c0 < 512
                          else [(c0, S - c0)])
                for (cs, w) in ntiles:
                    ssc_ps = pp3.tile([128, 512], F32, name="ssc_ps",
                                      tag="ssc", bufs=2)
                    nc.tensor.matmul(ssc_ps[:, :w], qT[h][:, c0:c0 + 128],
                                     kT[h][:, cs:cs + w], start=True, stop=True)
                    cnt_ps = pp3.tile([128, 512], F32, name="cnt_ps",
                                      tag="cnt", bufs=2)
                    nc.tensor.matmul(cnt_ps[:, :w], oh[h][0][:, c0:c0 + 128],
                                     oh[h][2][:, cs:cs + w], start=True, stop=False)
                    nc.tensor.matmul(cnt_ps[:, :w], oh[h][1][:, c0:c0 + 128],
                                     oh[h][3][:, cs:cs + w], start=False, stop=True)
                    # mask = sign(cnt) on ACT (counts >= 0), X = mask * ssc
                    cm = att.tile([128, 512], F16, name=f"cm_{h}_{bi}",
                                  tag="cm", bufs=3)
                    nc.scalar.sign(cm[:, :w], cnt_ps[:, :w])
                    nc.vector.tensor_mul(U[:, cs:cs + w], ssc_ps[:, :w],
                                         cm[:, :w])
                # diagonal block: A = triu*X + strict_lower*(X^T)
                xu = att.tile([128, 128], F16, name=f"xu_{h}_{bi}", tag="xu",
                              bufs=2)
                nc.gpsimd.affine_select(xu[:], U[:, c0:c0 + 128],
                                        pattern=[[1, 128]],
                                        compare_op=AOT.is_ge, fill=0.0,
                                        base=0, channel_multiplier=-1)
                dtr_ps = pp3.tile([128, 128], F16, name="dtr_ps", tag="dtr",
                                  bufs=1)
                nc.tensor.transpose(dtr_ps[:], U[:, c0:c0 + 128], ident16[:])
                nc.vector.select(U[:, c0:c0 + 128], slow[:], dtr_ps[:], xu[:])
            # pass A2: exp of upper strips (incl diag)
            for bi in range(NBI):
                c0 = bi * 128
                nc.scalar.activation(E[bi][:, c0:], Us[bi][:, c0:], ACT.Exp,
                                     scale=0.125,
                                     accum_out=zparts[:, bi * 8:bi * 8 + 1])
            # pass B: lower blocks via PE transposes, one bank + exp per row
            for bj in range(1, NBI):
                ltr_ps = pp3.tile([128, 896], F16, name="ltr_ps", tag="ltr",
                                  bufs=2)
                for bi2 in range(bj):
                    nc.tensor.transpose(
                        ltr_ps[:, bi2 * 128:bi2 * 128 + 128],
                        Us[bi2][:, bj * 128:bj * 128 + 128], ident16[:])
                nc.scalar.activation(
                    E[bj][:, 0:bj * 128], ltr_ps[:, 0:bj * 128],
                    ACT.Exp, scale=0.125,
                    accum_out=zparts[:, bj * 8 + 1:bj * 8 + 2])
            # Z and reciprocal
            zsum = att.tile([128, NBI], F32, name=f"zs_{h}", tag="zs", bufs=2)
            nc.vector.tensor_reduce(zsum[:],
                                    zparts[:].rearrange("p (b s) -> p b s", s=8),
                                    axis=mybir.AxisListType.X, op=AOT.add)
            nc.vector.reciprocal(zr[:], zsum[:])
            # PV
            for bi in range(NBI):
                pv_ps = pp3.tile([128, 64], F32, name="pv_ps", tag="pv", bufs=1)
                for bj in range(NBI):
                    nc.tensor.matmul(pv_ps[:], E[bj][:, bi * 128:bi * 128 + 128],
                                     vN[h][:, bj * 64:bj * 64 + 64],
                                     start=(bj == 0), stop=(bj == NBI - 1))
                outt = att.tile([128, 64], F32, name=f"o_{h}_{bi}", tag="outt",
                                bufs=3)
                nc.vector.tensor_scalar(out=outt[:], in0=pv_ps[:],
                                        scalar1=zr[:, bi:bi + 1], scalar2=None,
                                        op0=AOT.mult)
                nc.sync.dma_start(aps["out"][h, bi * 128:bi * 128 + 128, :],
                                  outt[:])


def build_program(repeat=None):
    nc = bacc.Bacc("TRN2", target_bir_lowering=False, debug=False,
                   num_devices=N_CORES)
    aps = _declare_io(nc)
    with tile.TileContext(nc) as tc:
        if repeat is not None and repeat > 1:
            with tc.For_i(0, repeat, 1):
                with ExitStack() as ctx:
                    _emit_body(tc, aps, ctx)
        else:
            with ExitStack() as ctx:
                _emit_body(tc, aps, ctx)
    nc.compile()
    return nc


# --------------------------------------------------------------------------
# PJRT runner (cached jit)
# --------------------------------------------------------------------------

def _make_runner(nc, n_cores=N_CORES):
    import jax
    from jax.sharding import Mesh, PartitionSpec
    from jax.experimental.shard_map import shard_map
    from concourse import bass2jax
    from concourse.bass2jax import _bass_exec_p, install_neuronx_cc_hook

    install_neuronx_cc_hook()
    partition_name = (nc.partition_id_tensor.name
                      if nc.partition_id_tensor else None)
    in_names, out_names, out_avals, zero_shapes = [], [], [], []
    for alloc in nc.m.functions[0].allocations:
        if not isinstance(alloc, mybir.MemoryLocationSet):
            continue
        name = alloc.memorylocations[0].name
        if alloc.kind == "ExternalInput":
            if name != partition_name:
                in_names.append(name)
        elif alloc.kind == "ExternalOutput":
            out_names.append(name)
            shape = tuple(alloc.tensor_shape)
            dtype = mybir.dt.np(alloc.dtype)
            out_avals.append(jax.core.ShapedArray(shape, dtype))
            zero_shapes.append((shape, dtype))
    n_params = len(in_names)
    n_outs = len(out_names)
    all_in_names = list(in_names) + list(out_names)
    if partition_name is not None:
        all_in_names.append(partition_name)

    def _body(*args):
        operands = list(args)
        if partition_name is not None:
            operands.append(bass2jax.partition_id_tensor())
        outs = _bass_exec_p.bind(
            *operands,
            out_avals=tuple(out_avals),
            in_names=tuple(all_in_names),
            out_names=tuple(out_names),
            lowering_input_output_aliases=(),
            sim_require_finite=True,
            sim_require_nnan=True,
            nc=nc,
        )
        return tuple(outs)

    devices = jax.devices()[:n_cores]
    mesh = Mesh(np.asarray(devices), ("core",))
    in_specs = (PartitionSpec("core"),) * (n_params + n_outs)
    out_specs = (PartitionSpec("core"),) * n_outs
    sharded = jax.jit(
        shard_map(_body, mesh=mesh, in_specs=in_specs, out_specs=out_specs,
                  check_rep=False),
        keep_unused=True,
    )

    def run(in_maps):
        per_core = [[np.asarray(m[name]) for name in in_names] for m in in_maps]
        concat_in = [
            np.concatenate([per_core[c][i] for c in range(n_cores)], axis=0)
            for i in range(n_params)
        ]
        concat_zeros = [
            np.zeros((n_cores * sh[0], *sh[1:]), dt) for (sh, dt) in zero_shapes
        ]
        out_arrs = sharded(*concat_in, *concat_zeros)
        out_arrs = [np.asarray(o) for o in out_arrs]
        return [
            {name: out_arrs[i].reshape(n_cores, *out_avals[i].shape)[c]
             for i, name in enumerate(out_names)}
            for c in range(n_cores)
        ]

    return run


def _get_runner(repeat=None):
    key = ("runner", repeat)
    if key not in _CACHE:
        nc = build_program(repeat=repeat)
        _CACHE[key] = _make_runner(nc)
    return _CACHE[key]


# --------------------------------------------------------------------------
# public entry point
# --------------------------------------------------------------------------

def kernel(**inputs):
    in_maps = _host_prep(inputs)
    run = _get_runner()
    results = run(in_maps)
    out = np.zeros((B, S, D), np.float32)
    for c in range(N_CORES):
        b = c // 4
        for j, h in enumerate(_core_heads(c)):
            out[b, :, h * dh:(h + 1) * dh] = results[c]["out"][j]
    return out


# revision 25
# speedup vs baseline: 1.9004x; 1.9004x over previous
"""Bass/Trainium2 kernel for BertLSHSelfAttention (B=2, S=1024, D=768, H=12).

Self-contained: accepts FULL unsharded inputs, shards (batch, head) pairs
across 8 NeuronCores (3 heads per core), runs a Tile/Bass program per core,
and reassembles the full [B, S, D] output.

Per-core pipeline:
  1. fp16 QKV projections on TensorE (packed 576 output rows in 5 M-groups).
  2. fp32 LSH hash projections via host-folded rv_eff = rv @ W (sign fidelity),
     hash values via block-diag coeff matmul, mod 64 on VectorE.
  3. One-hot bucket encodings (bf16) via selector-matmul broadcast + is_equal.
  4. Collision counts = onehot_q @ onehot_k^T (bf16, exact), masked scores
     X = min(cnt,1) * (q @ k^T fp16), triu symmetrization via PE transposes.
  5. exp((1/8) A) on ScalarE with fused row-sum accumulation -> fp16 E.
  6. ctx = (E @ v) * (1/Z) with fp16 PV matmul (E symmetric => no transpose).
"""

import math
from contextlib import ExitStack

import numpy as np
import ml_dtypes

import concourse.bass as bass
import concourse.tile as tile
from concourse import bacc, mybir

B, S, D = 2, 1024, 768
H, NH, BANDS, TABLE = 12, 4, 8, 64
dh = D // H            # 64
N_CORES = 8
HPC = 3                # heads per core
KCH = D // 128         # 6 contraction chunks
NBI = S // 128         # 8 row blocks
F16 = mybir.dt.float16
F32 = mybir.dt.float32
BF16 = mybir.dt.bfloat16
AOT = mybir.AluOpType
ACT = mybir.ActivationFunctionType

_CACHE = {}


# --------------------------------------------------------------------------
# host-side prep
# --------------------------------------------------------------------------

def _core_heads(c):
    return [(c % 4) * HPC + j for j in range(HPC)]


def _host_prep(inputs):
    """Build per-core input maps (numpy) for the SPMD program."""
    hs = np.asarray(inputs["hidden_states"], np.float32)
    wq = np.asarray(inputs["w_q"], np.float32)
    wk = np.asarray(inputs["w_k"], np.float32)
    wv = np.asarray(inputs["w_v"], np.float32)
    bq = np.asarray(inputs["b_q"], np.float32)
    bk = np.asarray(inputs["b_k"], np.float32)
    bv = np.asarray(inputs["b_v"], np.float32)
    rv = np.asarray(inputs["random_vectors"], np.float32)   # [B,H,NH,BANDS,dh]
    co = np.asarray(inputs["hash_coeffs"])                  # [B,H,NH,BANDS] int32

    # constants shared by all cores
    e8 = np.zeros((8, 4 * 128), ml_dtypes.bfloat16)
    for j in range(4):
        for p in range(128):
            e8[2 * j + p // 64, j * 128 + p] = 1.0
    e8_rep = np.zeros((72, 4 * 128), ml_dtypes.bfloat16)
    for base in (0, 32, 64):
        e8_rep[base:base + 8] = e8
    e8 = e8_rep
    iota2 = (np.arange(128) % 64).astype(np.float32)[:, None]
    pp, ff = np.mgrid[0:128, 0:128]
    triu = (pp <= ff).astype(np.float16)
    slow = (pp > ff).astype(np.uint8)
    ident32 = np.eye(128, dtype=np.float32)
    ident16 = np.eye(128, dtype=np.float16)

    in_maps = []
    for c in range(N_CORES):
        b = c // 4
        heads = _core_heads(c)
        hT = hs[b].T.copy()                                  # [768, 1024]
        # packed [128, KCH*S]: col block k holds rows k*128..k*128+127
        hT16 = np.ascontiguousarray(
            hT.astype(np.float16).reshape(KCH, 128, S).transpose(1, 0, 2)
        ).reshape(128, KCH * S)
        hT32 = np.ascontiguousarray(
            hT.astype(np.float32).reshape(KCH, 128, S).transpose(1, 0, 2)
        ).reshape(128, KCH * S)

        # packed projection weights: rows = per-head [q(64) k(64) v(64)]
        W_all = np.concatenate(
            [w[h * dh:(h + 1) * dh] for h in heads for w in (wq, wk, wv)], 0)
        b_all = np.concatenate(
            [bb[h * dh:(h + 1) * dh] for h in heads for bb in (bq, bk, bv)], 0)
        W_pad = np.zeros((640, D), np.float32)
        W_pad[:576] = W_all
        b_pad = np.zeros(640, np.float32)
        b_pad[:576] = b_all
        lhsT = W_pad.T                                       # [768, 640]
        # packed [128, 5*KCH*128]: col block (g*KCH+k)*128 holds chunk (g,k)
        w16 = np.zeros((128, 5 * KCH * 128), np.float16)
        for g in range(5):
            for k in range(KCH):
                w16[:, (g * KCH + k) * 128:(g * KCH + k + 1) * 128] = \
                    lhsT[k * 128:(k + 1) * 128, g * 128:(g + 1) * 128]
        b_pack = np.ascontiguousarray(b_pad.reshape(5, 128).T).astype(np.float32)

        # hash-projection weights: rv folded through W in float64
        rveff = np.zeros((128, 2 * KCH * 128), np.float32)
        rvbias = np.zeros((128, 2), np.float32)
        cb = np.zeros((128, HPC * 32), ml_dtypes.bfloat16)
        for slot, h in enumerate(heads):
            g2, half = slot // 2, slot % 2
            rq = np.einsum("nkd,dc->nkc", rv[b, h].astype(np.float64),
                           wq[h * dh:(h + 1) * dh].astype(np.float64))
            rk = np.einsum("nkd,dc->nkc", rv[b, h].astype(np.float64),
                           wk[h * dh:(h + 1) * dh].astype(np.float64))
            blk = np.concatenate([rq.reshape(32, D), rk.reshape(32, D)], 0)
            blkT = blk.T.astype(np.float32).reshape(KCH, 128, 64)
            for k in range(KCH):
                rveff[:, (g2 * KCH + k) * 128 + half * 64:
                      (g2 * KCH + k) * 128 + half * 64 + 64] = blkT[k]
            bq_f = np.einsum("nkd,d->nk", rv[b, h].astype(np.float64),
                             bq[h * dh:(h + 1) * dh].astype(np.float64)).ravel()
            bk_f = np.einsum("nkd,d->nk", rv[b, h].astype(np.float64),
                             bk[h * dh:(h + 1) * dh].astype(np.float64)).ravel()
            rvbias[half * 64:half * 64 + 32, g2] = bq_f.astype(np.float32)
            rvbias[half * 64 + 32:half * 64 + 64, g2] = bk_f.astype(np.float32)
            for n in range(NH):
                for band in range(BANDS):
                    r0 = half * 64
                    cval = float(co[b, h, n, band])
                    cb[r0 + n * 8 + band, slot * 32 + n] = cval        # hq
                    cb[r0 + 32 + n * 8 + band, slot * 32 + 4 + n] = cval  # hk

        in_maps.append({
            "hT16": hT16, "hT32": hT32,
            "w16": w16, "b_pack": b_pack,
            "rveff": rveff, "rvbias": rvbias, "cb": cb,
            "e8": e8, "iota2": iota2, "triu": triu, "slow": slow,
            "ident32": ident32, "ident16": ident16,
        })
    return in_maps


# --------------------------------------------------------------------------
# device program
# --------------------------------------------------------------------------

def _declare_io(nc):
    aps = {}
    decl = [
        ("hT16", [128, KCH * S], F16), ("hT32", [128, KCH * S], F32),
        ("w16", [128, 5 * KCH * 128], F16), ("b_pack", [128, 5], F32),
        ("rveff", [128, 2 * KCH * 128], F32), ("rvbias", [128, 2], F32),
        ("cb", [128, HPC * 32], BF16), ("e8", [72, 4 * 128], BF16),
        ("iota2", [128, 1], F32), ("triu", [128, 128], F16),
        ("slow", [128, 128], mybir.dt.uint8), ("ident32", [128, 128], F32),
        ("ident16", [128, 128], F16),
    ]
    for name, shape, dt in decl:
        aps[name] = nc.dram_tensor(name, shape, dt, kind="ExternalInput").ap()
    aps["out"] = nc.dram_tensor("out", [HPC, S, dh], F32,
                                kind="ExternalOutput").ap()
    return aps


def _emit_body(tc, aps, ctx):
    nc = tc.nc
    const = ctx.enter_context(tc.tile_pool(name="const", bufs=1))
    sbuf = ctx.enter_context(tc.tile_pool(name="sbuf", bufs=1))

    # ---- constants / inputs to SBUF ----
    def load(name, shape, dt, src):
        t = const.tile(shape, dt, name=name)
        nc.sync.dma_start(t[:], src)
        return t

    hT16_t = load("hT16_t", [128, KCH * S], F16, aps["hT16"][:])
    w16_t = load("w16_t", [128, 5 * KCH * 128], F16, aps["w16"][:])
    hT32_t = load("hT32_t", [128, KCH * S], F32, aps["hT32"][:])
    rveff_t = load("rveff_t", [128, 2 * KCH * 128], F32, aps["rveff"][:])
    hT16 = [hT16_t[:, k * S:(k + 1) * S] for k in range(KCH)]
    hT32 = [hT32_t[:, k * S:(k + 1) * S] for k in range(KCH)]
    w16 = [[w16_t[:, (g * KCH + k) * 128:(g * KCH + k + 1) * 128]
            for k in range(KCH)] for g in range(5)]
    rveff = [[rveff_t[:, (g * KCH + k) * 128:(g * KCH + k + 1) * 128]
              for k in range(KCH)] for g in range(2)]
    b_pack_t = load("b_pack_t", [128, 5], F32, aps["b_pack"][:])
    b_pack = [b_pack_t[:, g:g + 1] for g in range(5)]
    rvbias_t = load("rvbias_t", [128, 2], F32, aps["rvbias"][:])
    rvbias = [rvbias_t[:, g:g + 1] for g in range(2)]
    cb_t = load("cb_t", [128, HPC * 32], BF16, aps["cb"][:])
    e8_t = load("e8_t", [72, 4 * 128], BF16, aps["e8"][:])
    cb = [cb_t[:, j * 32:(j + 1) * 32] for j in range(HPC)]
    iota2 = load("iota2", [128, 1], F32, aps["iota2"][:])
    triu = load("triu", [128, 128], F16, aps["triu"][:])
    slow = load("slow", [128, 128], mybir.dt.uint8, aps["slow"][:])
    ident32 = load("ident32", [128, 128], F32, aps["ident32"][:])
    ident16 = load("ident16", [128, 128], F16, aps["ident16"][:])

    # ---- persistent per-head tensors ----
    qT = [sbuf.tile([64, S], F16, name=f"qT_{h}") for h in range(HPC)]
    kT = [sbuf.tile([64, S], F16, name=f"kT_{h}") for h in range(HPC)]
    vT = [sbuf.tile([64, S], F16, name=f"vT_{h}") for h in range(HPC)]
    vN = [sbuf.tile([128, NBI * dh], F16, name=f"vN_{h}") for h in range(HPC)]
    bits = [sbuf.tile([128, S], BF16, name=f"bits_{g}") for g in range(2)]
    hqi = sbuf.tile([96, S], mybir.dt.int32, name="hqi")
    hqa = sbuf.tile([96, S], mybir.dt.int32, name="hqa")
    hq8_all = sbuf.tile([96, S], BF16, name="hq8_all")
    # one-hot chunks per head: [qA, qB, kA, kB]
    oh = [[sbuf.tile([128, S], BF16, name=f"oh_{h}_{j}") for j in range(4)]
          for h in range(HPC)]

    # ---- phase 1: QKV projections (fp16) ----
    # group g covers packed rows [g*128, g*128+128); row layout per head:
    # q(64) k(64) v(64).  seg targets: (tile, tile_row_base)
    seg_map = {
        0: [(qT[0], 0), (kT[0], 0)],
        1: [(vT[0], 0), (qT[1], 0)],
        2: [(kT[1], 0), (vT[1], 0)],
        3: [(qT[2], 0), (kT[2], 0)],
        4: [(vT[2], 0), (None, 0)],
    }
    with tc.tile_pool(name="psum1", bufs=2, space="PSUM") as pp1, \
         tc.tile_pool(name="psum2", bufs=2, space="PSUM") as pp2:
        for g in range(5):
            for nh2 in range(2):
                c0 = nh2 * 512
                proj_ps = pp1.tile([128, 512], F32, name="proj_ps")
                for k in range(KCH):
                    nc.tensor.matmul(proj_ps[:], w16[g][k][:],
                                     hT16[k][:, c0:c0 + 512],
                                     start=(k == 0), stop=(k == KCH - 1))
                for half in range(2):
                    tgt, _ = seg_map[g][half]
                    if tgt is None:
                        continue
                    nc.scalar.activation(
                        tgt[:, c0:c0 + 512],
                        proj_ps[half * 64:half * 64 + 64, :],
                        ACT.Identity,
                        bias=b_pack[g][half * 64:half * 64 + 64, :])
        # v natural layout via PE transpose
        for h in range(HPC):
            for b4 in range(NBI // 4):
                vtr_ps = pp1.tile([128, 256], F16, name="vtr_ps", bufs=2)
                for q in range(4):
                    bi = b4 * 4 + q
                    nc.tensor.transpose(vtr_ps[:, q * 64:q * 64 + 64],
                                        vT[h][:, bi * 128:bi * 128 + 128],
                                        ident16[0:64, 0:64])
                nc.vector.tensor_copy(vN[h][:, b4 * 256:b4 * 256 + 256],
                                      vtr_ps[:])

        # ---- phase 2: hashes ----
        for g in range(2):
            for nh2 in range(2):
                c0 = nh2 * 512
                hash_ps = pp2.tile([128, 512], F32, name="hash_ps")
                for k in range(KCH):
                    nc.tensor.matmul(hash_ps[:], rveff[g][k][:],
                                     hT32[k][:, c0:c0 + 512],
                                     start=(k == 0), stop=(k == KCH - 1))
                nc.vector.tensor_scalar(
                    out=bits[g][:, c0:c0 + 512], in0=hash_ps[:],
                    scalar1=rvbias[g][:], scalar2=0.0,
                    op0=AOT.add, op1=AOT.is_gt)
        hq_ps = pp2.tile([96, S], F32, name="hq_ps", bufs=1)
        for slot in range(HPC):
            g2 = slot // 2
            for nh2 in range(2):
                c0 = nh2 * 512
                nc.tensor.matmul(hq_ps[32 * slot:32 * slot + 32, c0:c0 + 512],
                                 cb[slot][:], bits[g2][:, c0:c0 + 512],
                                 start=True, stop=True)
        # hash mod 64 == int bitwise AND with 63 (raw hash is an exact int)
        nc.vector.tensor_copy(hqi[:], hq_ps[:])
        nc.vector.tensor_scalar(out=hqa[:], in0=hqi[:], scalar1=63,
                                scalar2=None, op0=AOT.bitwise_and)
        nc.vector.tensor_copy(hq8_all[:], hqa[:])
    # ---- phase 3: attention per head ----
    att = ctx.enter_context(tc.tile_pool(name="att", bufs=1))
    with tc.tile_pool(name="psum3", bufs=1, space="PSUM") as pp3:
        for h in range(HPC):
            # one-hot encodings: selector-matmul broadcast + DVE is_equal
            for j in range(4):
                for nh2 in range(2):
                    c0 = nh2 * 512
                    bc_ps = pp3.tile([128, 512], F32, name="bc_ps", tag="ssc",
                                     bufs=2)
                    nc.tensor.matmul(
                        bc_ps[:],
                        e8_t[32 * h:32 * h + 8, j * 128:(j + 1) * 128],
                        hq8_all[32 * h:32 * h + 8, c0:c0 + 512],
                        start=True, stop=True)
                    nc.vector.tensor_scalar(
                        out=oh[h][j][:, c0:c0 + 512], in0=bc_ps[:],
                        scalar1=iota2[:], scalar2=None, op0=AOT.is_equal)
            E = [att.tile([128, S], F16, name=f"E_{h}_{bi}", tag=f"E{bi}",
                          bufs=2) for bi in range(NBI)]
            Us = [att.tile([128, S], F16, name=f"U_{h}_{bi}", tag=f"U{bi}",
                           bufs=2) for bi in range(NBI)]
            zparts = att.tile([128, 64], F32, name=f"zp_{h}", tag="zp", bufs=2)
            nc.vector.memset(zparts[:], 0.0)
            zr = att.tile([128, NBI], F32, name=f"zr_{h}", tag="zr", bufs=2)

            # pass A: masked upper strips + diag + upper exp
            for bi in range(NBI):
                U = Us[bi]
                c0 = bi * 128
                ntiles = ([(c0, 512 - c0), (512, 512)] if c0 < 512
                          else [(c0, S - c0)])
                for (cs, w) in ntiles:
                    ssc_ps = pp3.tile([128, 512], F32, name="ssc_ps",
                                      tag="ssc", bufs=2)
                    nc.tensor.matmul(ssc_ps[:, :w], qT[h][:, c0:c0 + 128],
                                     kT[h][:, cs:cs + w], start=True, stop=True)
                    cnt_ps = pp3.tile([128, 512], F32, name="cnt_ps",
                                      tag="cnt", bufs=2)
                    nc.tensor.matmul(cnt_ps[:, :w], oh[h][0][:, c0:c0 + 128],
                                     oh[h][2][:, cs:cs + w], start=True, stop=False)
                    nc.tensor.matmul(cnt_ps[:, :w], oh[h][1][:, c0:c0 + 128],
                                     oh[h][3][:, cs:cs + w], start=False, stop=True)
                    # mask = sign(cnt) on ACT (counts >= 0), X = mask * ssc
                    cm = att.tile([128, 512], F16, name=f"cm_{h}_{bi}",
                                  tag="cm", bufs=3)
                    nc.scalar.sign(cm[:, :w], cnt_ps[:, :w])
                    nc.vector.tensor_mul(U[:, cs:cs + w], ssc_ps[:, :w],
                                         cm[:, :w])
                # diagonal block: A = triu*X + strict_lower*(X^T)
                xu = att.tile([128, 128], F16, name=f"xu_{h}_{bi}", tag="xu",
                              bufs=2)
                nc.gpsimd.affine_select(xu[:], U[:, c0:c0 + 128],
                                        pattern=[[1, 128]],
                                        compare_op=AOT.is_ge, fill=0.0,
                                        base=0, channel_multiplier=-1)
                dtr_ps = pp3.tile([128, 128], F16, name="dtr_ps", tag="dtr",
                                  bufs=1)
                nc.tensor.transpose(dtr_ps[:], U[:, c0:c0 + 128], ident16[:])
                nc.vector.select(U[:, c0:c0 + 128], slow[:], dtr_ps[:], xu[:])
            # pass A2: exp of upper strips (incl diag)
            for bi in range(NBI):
                c0 = bi * 128
                nc.scalar.activation(E[bi][:, c0:], Us[bi][:, c0:], ACT.Exp,
                                     scale=0.125,
                                     accum_out=zparts[:, bi * 8:bi * 8 + 1])
            # pass B: lower blocks via PE transposes, one bank + exp per row
            for bj in range(1, NBI):
                ltr_ps = pp3.tile([128, 896], F16, name="ltr_ps", tag="ltr",
                                  bufs=2)
                for bi2 in range(bj):
                    nc.tensor.transpose(
                        ltr_ps[:, bi2 * 128:bi2 * 128 + 128],
                        Us[bi2][:, bj * 128:bj * 128 + 128], ident16[:])
                nc.scalar.activation(
                    E[bj][:, 0:bj * 128], ltr_ps[:, 0:bj * 128],
                    ACT.Exp, scale=0.125,
                    accum_out=zparts[:, bj * 8 + 1:bj * 8 + 2])
            # Z and reciprocal
            zsum = att.tile([128, NBI], F32, name=f"zs_{h}", tag="zs", bufs=2)
            nc.vector.tensor_reduce(zsum[:],
                                    zparts[:].rearrange("p (b s) -> p b s", s=8),
                                    axis=mybir.AxisListType.X, op=AOT.add)
            nc.vector.reciprocal(zr[:], zsum[:])
            # PV
            for bi in range(NBI):
                pv_ps = pp3.tile([128, 64], F32, name="pv_ps", tag="pv", bufs=1)
                for bj in range(NBI):
                    nc.tensor.matmul(pv_ps[:], E[bj][:, bi * 128:bi * 128 + 128],
                                     vN[h][:, bj * 64:bj * 64 + 64],
                                     start=(bj == 0), stop=(bj == NBI - 1))
                outt = att.tile([128, 64], F32, name=f"o_{h}_{bi}", tag="outt",
                                bufs=3)
                nc.vector.tensor_scalar(out=outt[:], in0=pv_ps[:],
                                        scalar1=zr[:, bi:bi + 1], scalar2=None,
                                        op0=AOT.mult)
                nc.sync.dma_start(aps["out"][h, bi * 128:bi * 128 + 128, :],
                                  outt[:])


def build_program(repeat=None):
    nc = bacc.Bacc("TRN2", target_bir_lowering=False, debug=False,
                   num_devices=N_CORES)
    aps = _declare_io(nc)
    with tile.TileContext(nc) as tc:
        if repeat is not None and repeat > 1:
            with tc.For_i(0, repeat, 1):
                with ExitStack() as ctx:
                    _emit_body(tc, aps, ctx)
        else:
            with ExitStack() as ctx:
                _emit_body(tc, aps, ctx)
    nc.compile()
    return nc


# --------------------------------------------------------------------------
# PJRT runner (cached jit)
# --------------------------------------------------------------------------

def _make_runner(nc, n_cores=N_CORES):
    import jax
    from jax.sharding import Mesh, PartitionSpec
    from jax.experimental.shard_map import shard_map
    from concourse import bass2jax
    from concourse.bass2jax import _bass_exec_p, install_neuronx_cc_hook

    install_neuronx_cc_hook()
    partition_name = (nc.partition_id_tensor.name
                      if nc.partition_id_tensor else None)
    in_names, out_names, out_avals, zero_shapes = [], [], [], []
    for alloc in nc.m.functions[0].allocations:
        if not isinstance(alloc, mybir.MemoryLocationSet):
            continue
        name = alloc.memorylocations[0].name
        if alloc.kind == "ExternalInput":
            if name != partition_name:
                in_names.append(name)
        elif alloc.kind == "ExternalOutput":
            out_names.append(name)
            shape = tuple(alloc.tensor_shape)
            dtype = mybir.dt.np(alloc.dtype)
            out_avals.append(jax.core.ShapedArray(shape, dtype))
            zero_shapes.append((shape, dtype))
    n_params = len(in_names)
    n_outs = len(out_names)
    all_in_names = list(in_names) + list(out_names)
    if partition_name is not None:
        all_in_names.append(partition_name)

    def _body(*args):
        operands = list(args)
        if partition_name is not None:
            operands.append(bass2jax.partition_id_tensor())
        outs = _bass_exec_p.bind(
            *operands,
            out_avals=tuple(out_avals),
            in_names=tuple(all_in_names),
            out_names=tuple(out_names),
            lowering_input_output_aliases=(),
            sim_require_finite=True,
            sim_require_nnan=True,
            nc=nc,
        )
        return tuple(outs)

    devices = jax.devices()[:n_cores]
    mesh = Mesh(np.asarray(devices), ("core",))
    in_specs = (PartitionSpec("core"),) * (n_params + n_outs)
    out_specs = (PartitionSpec("core"),) * n_outs
    sharded = jax.jit(
        shard_map(_body, mesh=mesh, in_specs=in_specs, out_specs=out_specs,
                  check_rep=False),
        keep_unused=True,
    )

    def _unpack(out_arrs):
        out_arrs = [np.asarray(o) for o in out_arrs]
        return [
            {name: out_arrs[i].reshape(n_cores, *out_avals[i].shape)[c]
             for i, name in enumerate(out_names)}
            for c in range(n_cores)
        ]

    def prepare(in_maps):
        per_core = [[np.asarray(m[name]) for name in in_names] for m in in_maps]
        concat_in = [
            np.concatenate([per_core[c][i] for c in range(n_cores)], axis=0)
            for i in range(n_params)
        ]
        return [jax.device_put(a) for a in concat_in]

    def run_prepared(dev_in):
        concat_zeros = [
            np.zeros((n_cores * sh[0], *sh[1:]), dt) for (sh, dt) in zero_shapes
        ]
        return _unpack(sharded(*dev_in, *concat_zeros))

    def run(in_maps):
        return run_prepared(prepare(in_maps))

    run.prepare = prepare
    run.run_prepared = run_prepared
    return run


def _get_runner(repeat=None):
    key = ("runner", repeat)
    if key not in _CACHE:
        nc = build_program(repeat=repeat)
        _CACHE[key] = _make_runner(nc)
    return _CACHE[key]


# --------------------------------------------------------------------------
# public entry point
# --------------------------------------------------------------------------

def kernel(**inputs):
    in_maps = _host_prep(inputs)
    run = _get_runner()
    results = run(in_maps)
    out = np.zeros((B, S, D), np.float32)
    for c in range(N_CORES):
        b = c // 4
        for j, h in enumerate(_core_heads(c)):
            out[b, :, h * dh:(h + 1) * dh] = results[c]["out"][j]
    return out
